# revision 7
# baseline (speedup 1.0000x reference)
"""Trainium2 Bass kernel for nn_BayesianLayer (Bayesian linear layer).

Math (per batch row b):
    sigma      = softplus(ro)                          # (IN, OUT)
    weights_b  = eps_b * sigma + mu                    # (IN, OUT)
    bias_b     = eps_bias_b * softplus(ro_bias) + mu_bias
    out_b      = x_b @ weights_b + bias_b              # (OUT,)

Sharding: data-parallel over the batch dim across 8 NeuronCores
(16 rows each); mu/ro/biases replicated.

Per-core device kernel:
  - sigma = softplus(ro) computed on-chip (ACT Abs/Exp/Ln compose + DVE).
  - stream eps in [128, CHUNK_F] tiles (i on partitions), multiply by sigma
    on VectorE (output rounded to float32r), then reduce over i with
    TensorE GEMVs at full PE rate: the x column for (b, k-block) is the
    stationary operand, accumulated over k into PSUM. mu matmuls are
    folded into the same PSUM accumulation groups.
  - the 16 GEMV results live in one [128, 4096] PSUM tile: batch row b
    maps to partition 32*(b%4) (PE col-group tiling), free offset
    (b//4)*1024.
  - bias rows are precomputed, scattered to matching partitions via DMA,
    added to PSUM on VectorE, and DMA'd out.
"""

import numpy as np
from contextlib import ExitStack

import concourse.bass as bass
import concourse.mybir as mybir
import concourse.tile as tile
from concourse import bacc
from concourse.bass_utils import run_bass_kernel_spmd

B, IN, OUT = 128, 1024, 1024
N_CORES = 8
BP = B // N_CORES          # 16 batch rows per core
P = 128                    # partitions
KB = IN // P               # 8 k-blocks
NHALF = 512                # fp32-family matmul max moving free dim
import os
CHUNK_K = int(os.environ.get("BK_CHUNK_K", "2"))  # k-blocks per eps chunk
CHUNK_F = CHUNK_K * OUT    # free elems per chunk
N_CHUNKS = KB // CHUNK_K

f32 = mybir.dt.float32
f32r = mybir.dt.float32r
MULT = mybir.AluOpType.mult
ADD = mybir.AluOpType.add
MAX = mybir.AluOpType.max
ACT = mybir.ActivationFunctionType

EPS_BUFS = int(os.environ.get("BK_EPS_BUFS", "3"))

_compiled = {}


def _softplus_tiles(nc, out_sl, in_sl):
    """out_sl = softplus(in_sl) = ln(1 + exp(x)).

    Direct form: safe for |x| <~ 80 (inputs here are N(0,1)).
    """
    nc.scalar.activation(out_sl, in_sl, ACT.Exp)
    nc.scalar.activation(out_sl, out_sl, ACT.Ln, bias=1.0)


def build():
    nc = bacc.Bacc("TRN2", debug=False, enable_asserts=False)

    eps_d = nc.dram_tensor("eps", (BP, IN, OUT), f32, kind="ExternalInput").ap()
    xT_d = nc.dram_tensor("xT", (IN, BP), f32, kind="ExternalInput").ap()
    mu_d = nc.dram_tensor("mu", (IN, OUT), f32, kind="ExternalInput").ap()
    ro_d = nc.dram_tensor("ro", (IN, OUT), f32, kind="ExternalInput").ap()
    eb_d = nc.dram_tensor("ebias", (BP, OUT), f32, kind="ExternalInput").ap()
    rb_d = nc.dram_tensor("robias", (BP, OUT), f32, kind="ExternalInput").ap()
    mb_d = nc.dram_tensor("mubias", (BP, OUT), f32, kind="ExternalInput").ap()
    out_d = nc.dram_tensor("out", (BP, OUT), f32, kind="ExternalOutput").ap()

    # [p, k*OUT + o] layouts (i = k*128 + p on partitions)
    ro_r = ro_d.rearrange("(k p) o -> p k o", p=P)
    mu_r = mu_d.rearrange("(k p) o -> p k o", p=P)
    eps_r = eps_d.rearrange("b (k p) o -> b p k o", p=P)
    xT_r = xT_d.rearrange("(k p) m -> p k m", p=P)

    with tile.TileContext(nc) as tc, ExitStack() as ctx:
        consts = ctx.enter_context(tc.tile_pool(name="consts", bufs=1))
        small = ctx.enter_context(tc.tile_pool(name="small", bufs=1))
        eps_pool = ctx.enter_context(tc.tile_pool(name="eps_pool", bufs=EPS_BUFS))
        psum_pool = ctx.enter_context(tc.tile_pool(name="psum", bufs=1, space="PSUM"))

        # ---- constants / preamble ----
        # x columns first (tiny; needed by every matmul)
        xT_tmp = small.tile([P, KB, BP], f32)
        nc.sync.dma_start(xT_tmp[:], xT_r)
        xT_sb = consts.tile([P, KB, BP], f32r)
        nc.vector.tensor_copy(xT_sb[:], xT_tmp[:])

        # sigma (softplus on ACT) and mu (rounded to f32r on GpSimd),
        # interleaved per k-block so chunk 0 is ready as early as possible
        sigma_all = consts.tile([P, KB, OUT], f32)
        mu_all = consts.tile([P, KB, OUT], f32r)
        for c in range(KB):
            ro_t = small.tile([P, OUT], f32, tag="pre_tmp", bufs=4, name="ro_t")
            nc.sync.dma_start(ro_t[:], ro_r[:, c, :])
            _softplus_tiles(nc, sigma_all[:, c, :], ro_t[:])
            mu_t = small.tile([P, OUT], f32, tag="pre_tmp", bufs=4, name="mu_t")
            nc.sync.dma_start(mu_t[:], mu_r[:, c, :])
            nc.gpsimd.tensor_copy(mu_all[:, c, :], mu_t[:])

        # ---- bias rows: bias16 = ebias * softplus(robias) + mubias ----
        eb16 = small.tile([BP, OUT], f32)
        nc.sync.dma_start(eb16[:], eb_d)
        rb16 = small.tile([BP, OUT], f32)
        nc.sync.dma_start(rb16[:], rb_d)
        mb16 = small.tile([BP, OUT], f32)
        nc.sync.dma_start(mb16[:], mb_d)
        sb16 = small.tile([BP, OUT], f32)
        _softplus_tiles(nc, sb16[:], rb16[:])
        nc.vector.tensor_tensor(eb16[:], eb16[:], sb16[:], MULT)
        nc.vector.tensor_tensor(eb16[:], eb16[:], mb16[:], ADD)

        # ---- main loop ----
        for b in range(BP):
            prow = psum_pool.tile([1, OUT], f32, tag="pb", bufs=4, name="prow")
            for c in range(N_CHUNKS):
                ksl = slice(c * CHUNK_K, (c + 1) * CHUNK_K)
                et = eps_pool.tile(
                    [P, CHUNK_K, OUT], f32, tag="eps_t", name="et", bufs=EPS_BUFS + 2
                )
                dma_eng = nc.sync if (b * N_CHUNKS + c) % 2 == 0 else nc.scalar
                dma_eng.dma_start(et[:], eps_r[b][:, ksl, :])
                er = eps_pool.tile([P, CHUNK_K, OUT], f32r, tag="eps_r", name="er")
                nc.vector.tensor_tensor(er[:], et[:], sigma_all[:, ksl, :], MULT)
                for ks in range(CHUNK_K):
                    k = c * CHUNK_K + ks
                    lhsT = xT_sb[:, k, b : b + 1]
                    for h in range(2):
                        pr = prow[:, h * NHALF : (h + 1) * NHALF]
                        nc.tensor.matmul(
                            pr,
                            lhsT,
                            er[:, ks, h * NHALF : (h + 1) * NHALF],
                            start=(k == 0),
                            stop=False,
                        )
                        nc.tensor.matmul(
                            pr,
                            lhsT,
                            mu_all[:, k, h * NHALF : (h + 1) * NHALF],
                            start=False,
                            stop=(k == KB - 1),
                        )
            # tail: out row = psum + bias (partition 0), then stream it out
            bias_b = eps_pool.tile([1, OUT], f32, tag="bias_b", bufs=3, name="bias_b")
            nc.sync.dma_start(bias_b[:], eb16[b : b + 1, :])
            out_b = eps_pool.tile([1, OUT], f32, tag="out_b", bufs=3, name="out_b")
            nc.vector.tensor_tensor(out_b[:], prow[:], bias_b[:], ADD)
            nc.sync.dma_start(out_d[b : b + 1, :], out_b[:])

    nc.compile()
    return nc


def get_nc():
    key = (CHUNK_K, EPS_BUFS)
    if key not in _compiled:
        _compiled[key] = build()
    return _compiled[key]


def make_in_maps(x, eps, eps_bias, mu, ro, mu_bias, ro_bias):
    x = np.ascontiguousarray(np.asarray(x, dtype=np.float32))
    eps = np.asarray(eps, dtype=np.float32)
    eps_bias = np.asarray(eps_bias, dtype=np.float32)
    mu = np.ascontiguousarray(np.asarray(mu, dtype=np.float32))
    ro = np.ascontiguousarray(np.asarray(ro, dtype=np.float32))
    mu_b = np.ascontiguousarray(
        np.broadcast_to(np.asarray(mu_bias, dtype=np.float32).reshape(1, OUT), (BP, OUT))
    )
    ro_b = np.ascontiguousarray(
        np.broadcast_to(np.asarray(ro_bias, dtype=np.float32).reshape(1, OUT), (BP, OUT))
    )
    in_maps = []
    for c in range(N_CORES):
        sl = slice(c * BP, (c + 1) * BP)
        in_maps.append(
            {
                "eps": np.ascontiguousarray(eps[sl]),
                "xT": np.ascontiguousarray(x[sl].T),
                "mu": mu,
                "ro": ro,
                "ebias": np.ascontiguousarray(eps_bias[sl]),
                "robias": ro_b,
                "mubias": mu_b,
            }
        )
    return in_maps


def run(trace=False, **inputs):
    nc = get_nc()
    in_maps = make_in_maps(**inputs)
    res = run_bass_kernel_spmd(
        nc, in_maps, core_ids=list(range(N_CORES)), trace=trace
    )
    out = np.concatenate([r["out"] for r in res.results], axis=0)
    return out, res


def kernel(**inputs) -> np.ndarray:
    out, _ = run(trace=False, **inputs)
    return out


# revision 10
# speedup vs baseline: 173.5434x; 173.5434x over previous
"""Trainium2 Bass kernel for nn_BayesianLayer (Bayesian linear layer).

Math (per batch row b):
    sigma      = softplus(ro)                          # (IN, OUT)
    weights_b  = eps_b * sigma + mu                    # (IN, OUT)
    bias_b     = eps_bias_b * softplus(ro_bias) + mu_bias
    out_b      = x_b @ weights_b + bias_b              # (OUT,)

Sharding: data-parallel over the batch dim across 8 NeuronCores
(16 rows each); mu/ro/biases replicated.

Per-core device kernel:
  - sigma = softplus(ro) computed on-chip (ACT Abs/Exp/Ln compose + DVE).
  - stream eps in [128, CHUNK_F] tiles (i on partitions), multiply by sigma
    on VectorE (output rounded to float32r), then reduce over i with
    TensorE GEMVs at full PE rate: the x column for (b, k-block) is the
    stationary operand, accumulated over k into PSUM. mu matmuls are
    folded into the same PSUM accumulation groups.
  - GEMV results accumulate in [1, 1024] PSUM tiles (4 live groups =
    8 banks); bias rows are added on VectorE and each output row is
    DMA'd out as soon as it is ready.
"""

import os

import numpy as np
from contextlib import ExitStack

import concourse.mybir as mybir
import concourse.tile as tile
from concourse import bacc
from concourse.bass_utils import run_bass_kernel_spmd

B, IN, OUT = 128, 1024, 1024
N_CORES = 8
BP = B // N_CORES          # 16 batch rows per core
P = 128                    # partitions
KB = IN // P               # 8 k-blocks
NHALF = 512                # fp32-family matmul max moving free dim
CHUNK_K = int(os.environ.get("BK_CHUNK_K", "1"))  # k-blocks per eps chunk
N_CHUNKS = KB // CHUNK_K

f32 = mybir.dt.float32
f32r = mybir.dt.float32r
MULT = mybir.AluOpType.mult
ADD = mybir.AluOpType.add
ACT = mybir.ActivationFunctionType

EPS_BUFS = int(os.environ.get("BK_EPS_BUFS", "8"))
BLK = int(os.environ.get("BK_BLK", "2"))
REP = int(os.environ.get("BK_REP", "1"))

_compiled = {}


def _softplus_tiles(nc, out_sl, in_sl):
    """out_sl = softplus(in_sl) = ln(1 + exp(x)).

    Direct form: safe for |x| <~ 80 (inputs here are N(0,1)).
    """
    nc.scalar.activation(out_sl, in_sl, ACT.Exp)
    nc.scalar.activation(out_sl, out_sl, ACT.Ln, bias=1.0)


def build(rep=None):
    rep = REP if rep is None else rep
    nc = bacc.Bacc("TRN2", debug=False, enable_asserts=False)

    eps_d = nc.dram_tensor("eps", (BP, IN, OUT), f32, kind="ExternalInput").ap()
    xT_d = nc.dram_tensor("xT", (IN, BP), f32, kind="ExternalInput").ap()
    mu_d = nc.dram_tensor("mu", (IN, OUT), f32, kind="ExternalInput").ap()
    ro_d = nc.dram_tensor("ro", (IN, OUT), f32, kind="ExternalInput").ap()
    eb_d = nc.dram_tensor("ebias", (BP, OUT), f32, kind="ExternalInput").ap()
    rb_d = nc.dram_tensor("robias", (BP, OUT), f32, kind="ExternalInput").ap()
    mb_d = nc.dram_tensor("mubias", (BP, OUT), f32, kind="ExternalInput").ap()
    out_d = nc.dram_tensor("out", (BP, OUT), f32, kind="ExternalOutput").ap()

    # [p, k*OUT + o] layouts (i = k*128 + p on partitions)
    ro_r = ro_d.rearrange("(k p) o -> p k o", p=P)
    mu_r = mu_d.rearrange("(k p) o -> p k o", p=P)
    eps_r = eps_d.rearrange("b (k p) o -> b p k o", p=P)
    xT_r = xT_d.rearrange("(k p) m -> p k m", p=P)

    with tile.TileContext(nc) as tc, ExitStack() as ctx:
        consts = ctx.enter_context(tc.tile_pool(name="consts", bufs=1))
        small = ctx.enter_context(tc.tile_pool(name="small", bufs=1))
        eps_pool = ctx.enter_context(tc.tile_pool(name="eps_pool", bufs=EPS_BUFS))
        psum_pool = ctx.enter_context(tc.tile_pool(name="psum", bufs=1, space="PSUM"))

        for _rep in range(rep):
            # ---- constants / preamble ----
            # x columns first (tiny; needed by every matmul)
            xT_tmp = small.tile([P, KB, BP], f32)
            nc.gpsimd.dma_start(xT_tmp[:], xT_r)
            xT_sb = consts.tile([P, KB, BP], f32r)
            nc.vector.tensor_copy(xT_sb[:], xT_tmp[:])

            # sigma (softplus on ACT) and mu (rounded to f32r on GpSimd),
            # interleaved per k-block so chunk 0 is ready as early as possible
            sigma_all = consts.tile([P, KB, OUT], f32)
            mu_all = consts.tile([P, KB, OUT], f32r)
            for c in range(KB):
                ro_t = small.tile([P, OUT], f32, tag="pre_tmp", bufs=6, name="ro_t")
                nc.gpsimd.dma_start(ro_t[:], ro_r[:, c, :])
                _softplus_tiles(nc, sigma_all[:, c, :], ro_t[:])
                mu_t = small.tile([P, OUT], f32, tag="pre_tmp", bufs=6, name="mu_t")
                nc.gpsimd.dma_start(mu_t[:], mu_r[:, c, :])
                nc.gpsimd.tensor_copy(mu_all[:, c, :], mu_t[:])

            # ---- bias rows: bias16 = ebias * softplus(robias) + mubias ----
            eb16 = small.tile([BP, OUT], f32)
            nc.gpsimd.dma_start(eb16[:], eb_d)
            rb16 = small.tile([BP, OUT], f32)
            nc.gpsimd.dma_start(rb16[:], rb_d)
            mb16 = small.tile([BP, OUT], f32)
            nc.gpsimd.dma_start(mb16[:], mb_d)
            sb16 = small.tile([BP, OUT], f32)
            _softplus_tiles(nc, sb16[:], rb16[:])
            nc.vector.tensor_tensor(eb16[:], eb16[:], sb16[:], MULT)
            nc.vector.tensor_tensor(eb16[:], eb16[:], mb16[:], ADD)

            # ---- main loop: blocks of 4 batch rows, chunk-major inside ----
            # (the first 4 eps multiplies only need sigma chunk 0, so sigma
            # production always stays ahead; 4 live PSUM groups = 8 banks)
            blocks = [
                list(range(s, min(s + BLK, BP))) for s in range(0, BP, BLK)
            ]
            for blk in blocks:
                prows = {
                    b: psum_pool.tile([1, OUT], f32, tag="pb", bufs=4, name="prow")
                    for b in blk
                }
                for c in range(N_CHUNKS):
                    ksl = slice(c * CHUNK_K, (c + 1) * CHUNK_K)
                    for b in blk:
                        et = eps_pool.tile(
                            [P, CHUNK_K, OUT], f32, tag="eps_t", name="et",
                            bufs=EPS_BUFS + 2,
                        )
                        dma_eng = nc.sync if (b + c) % 2 == 0 else nc.scalar
                        dma_eng.dma_start(et[:], eps_r[b][:, ksl, :])
                        er = eps_pool.tile(
                            [P, CHUNK_K, OUT], f32r, tag="eps_r", name="er"
                        )
                        nc.vector.tensor_tensor(
                            er[:], et[:], sigma_all[:, ksl, :], MULT
                        )
                        for ks in range(CHUNK_K):
                            k = c * CHUNK_K + ks
                            lhsT = xT_sb[:, k, b : b + 1]
                            for h in range(2):
                                pr = prows[b][:, h * NHALF : (h + 1) * NHALF]
                                nc.tensor.matmul(
                                    pr,
                                    lhsT,
                                    er[:, ks, h * NHALF : (h + 1) * NHALF],
                                    start=(k == 0),
                                    stop=False,
                                )
                                nc.tensor.matmul(
                                    pr,
                                    lhsT,
                                    mu_all[:, k, h * NHALF : (h + 1) * NHALF],
                                    start=False,
                                    stop=(k == KB - 1),
                                )
                # tails: out row = psum + bias (partition 0), stream out
                for b in blk:
                    bias_b = eps_pool.tile(
                        [1, OUT], f32, tag="bias_b", bufs=3, name="bias_b"
                    )
                    nc.gpsimd.dma_start(bias_b[:], eb16[b : b + 1, :])
                    out_b = eps_pool.tile(
                        [1, OUT], f32, tag="out_b", bufs=3, name="out_b"
                    )
                    nc.vector.tensor_tensor(out_b[:], prows[b][:], bias_b[:], ADD)
                    nc.sync.dma_start(out_d[b : b + 1, :], out_b[:])

    nc.compile()
    return nc


def get_nc(rep=None):
    rep = REP if rep is None else rep
    key = (CHUNK_K, EPS_BUFS, BLK, rep)
    if key not in _compiled:
        _compiled[key] = build(rep)
    return _compiled[key]


def make_in_maps(x, eps, eps_bias, mu, ro, mu_bias, ro_bias):
    x = np.ascontiguousarray(np.asarray(x, dtype=np.float32))
    eps = np.asarray(eps, dtype=np.float32)
    eps_bias = np.asarray(eps_bias, dtype=np.float32)
    mu = np.ascontiguousarray(np.asarray(mu, dtype=np.float32))
    ro = np.ascontiguousarray(np.asarray(ro, dtype=np.float32))
    mu_b = np.ascontiguousarray(
        np.broadcast_to(np.asarray(mu_bias, dtype=np.float32).reshape(1, OUT), (BP, OUT))
    )
    ro_b = np.ascontiguousarray(
        np.broadcast_to(np.asarray(ro_bias, dtype=np.float32).reshape(1, OUT), (BP, OUT))
    )
    in_maps = []
    for c in range(N_CORES):
        sl = slice(c * BP, (c + 1) * BP)
        in_maps.append(
            {
                "eps": np.ascontiguousarray(eps[sl]),
                "xT": np.ascontiguousarray(x[sl].T),
                "mu": mu,
                "ro": ro,
                "ebias": np.ascontiguousarray(eps_bias[sl]),
                "robias": ro_b,
                "mubias": mu_b,
            }
        )
    return in_maps


def run(trace=False, **inputs):
    nc = get_nc()
    in_maps = make_in_maps(**inputs)
    res = run_bass_kernel_spmd(
        nc, in_maps, core_ids=list(range(N_CORES)), trace=trace
    )
    out = np.concatenate([r["out"] for r in res.results], axis=0)
    return out, res


def kernel(**inputs) -> np.ndarray:
    out, _ = run(trace=False, **inputs)
    return out

